# revision 21
# baseline (speedup 1.0000x reference)
"""Trainium2 Bass kernel for multi-head attention (B=4, N=2048, DIM=1024, H=16, DH=64).

Sharding: 8 cores = 4 batches x 2 query-halves. Each core receives x[b]^T with
its query-half columns rotated to the front (attention is invariant to a
consistent permutation of the key/value axis), computes q for columns 0:1024,
k/v for all 2048, runs scores^T = k_h^T @ q_h per head (row-tiled pairs),
softmax via exp + ones-column denominator folded into the AV matmul, and the
full output projection for its rows. Outputs are disjoint across cores.

Emission interleaves the second half of the projections into the first head
pairs (PE executes in program order, so overlap must be authored), and the
output projection runs two-pass (heads 0-6 early, head pair 7 joined late) to
hide the final softmax-normalize latency.
"""

import os

import numpy as np
import ml_dtypes

import concourse.bass as bass
import concourse.tile as tile
from concourse import bacc, mybir
from concourse import bass_utils

B, N, DIM = 4, 2048, 1024
HEADS, DH = 16, 64
INNER = HEADS * DH
SCALE = DH ** -0.5
NCORES = 8
IH = N // 2          # query rows per core (i-half)
BF16 = mybir.dt.bfloat16
F32 = mybir.dt.float32

KT = DIM // 128          # 8 contraction tiles for projections
NT = N // 128            # 16 j tiles
ES = INNER // 128        # 8 e-slices for q or k
NPP = 8                  # ns groups computed two-pass in phase 3 (all)

_CACHE = {}


def _build_program():
    nc = bacc.Bacc("TRN2", target_bir_lowering=False, debug=False)

    xT_d = nc.dram_tensor("xT", [DIM, N], BF16, kind="ExternalInput")
    wqkv_d = nc.dram_tensor("w_qkv", [DIM, 3 * INNER], BF16, kind="ExternalInput")
    wout_d = nc.dram_tensor("w_out", [INNER, DIM], BF16, kind="ExternalInput")
    bout_d = nc.dram_tensor("b_out", [DIM], F32, kind="ExternalInput")
    out_d = nc.dram_tensor("out", [IH, DIM], F32, kind="ExternalOutput")

    with tile.TileContext(nc) as tc:
        _emit(tc, nc, xT_d, wqkv_d, wout_d, bout_d, out_d)
    nc.compile()
    return nc


def _emit(tc, nc, xT_d, wqkv_d, wout_d, bout_d, out_d):
    from contextlib import ExitStack

    xT_r = xT_d.ap().rearrange("(t p) n -> p t n", p=128)       # [128, 8, 2048]
    w_r = wqkv_d.ap().rearrange("(t p) e -> p t e", p=128)      # [128, 8, 3072]
    wo_r = wout_d.ap().rearrange("(t p) d -> p t d", p=128)     # [128, 8, 1024]

    bap = bout_d.ap()
    bias_bcast = bass.AP(tensor=bap.tensor, offset=bap.offset,
                         ap=[[0, 128]] + [list(d) for d in bap.ap])

    with ExitStack() as ctx:
        consts = ctx.enter_context(tc.tile_pool(name="consts", bufs=1))
        qkv_out = ctx.enter_context(tc.tile_pool(name="qkv_out", bufs=1))
        attn_out = ctx.enter_context(tc.tile_pool(name="attn_out", bufs=1))
        atp = ctx.enter_context(tc.tile_pool(name="attnT", bufs=4))
        rcp = ctx.enter_context(tc.tile_pool(name="rcp", bufs=1))
        bcsp = ctx.enter_context(tc.tile_pool(name="bcs", bufs=2))
        avup = ctx.enter_context(tc.tile_pool(name="avu", bufs=2))
        oddp = ctx.enter_context(tc.tile_pool(name="odd", bufs=1))
        drbp = ctx.enter_context(tc.tile_pool(name="drb", bufs=2, space="DRAM"))
        ps_sc = ctx.enter_context(tc.tile_pool(name="ps_sc", bufs=2, space="PSUM"))

        bias_sb = consts.tile([128, DIM], F32)
        nc.sync.dma_start(out=bias_sb, in_=bias_bcast)
        wo_sb = consts.tile([128, ES, DIM], BF16)       # head pair hp at [:, hp, :]
        nc.sync.dma_start(out=wo_sb, in_=wo_r)

        qTs = [qkv_out.tile([128, IH], BF16, name=f"qT{s}") for s in range(ES)]
        kTs = [qkv_out.tile([128, N], BF16, name=f"kT{s}") for s in range(ES)]
        v_lo = qkv_out.tile([128, NT, 8, DH + 1], BF16)  # heads 0-7 (+ones col)
        v_hi = qkv_out.tile([128, NT, 8, DH + 1], BF16)  # heads 8-15
        nc.vector.memset(v_lo[:, :, :, DH], 1.0)
        nc.vector.memset(v_hi[:, :, :, DH], 1.0)
        aoTs = [attn_out.tile([128, IH], BF16, name=f"aoT{s}") for s in range(ES)]

        # ---- projection group emitters (psum from a given pool/tag) ----
        def q_slice(pool, tag, wg, s4, s):
            ps = pool.tile([128, IH], F32, tag=tag, name=f"q_ps{s}")
            for c in range(IH // 512):
                for k in range(KT):
                    nc.tensor.matmul(
                        ps[:, 512 * c:512 * (c + 1)],
                        wg[:, k, 128 * s4:128 * (s4 + 1)],
                        xTk[k][:, 512 * c:512 * (c + 1)],
                        start=(k == 0), stop=(k == KT - 1))
            nc.vector.tensor_copy(out=qTs[s], in_=ps)

        def k_slice(pool, tag, wg, s4, s, half):
            ps = pool.tile([128, IH], F32, tag=tag, name=f"k_ps{s}_{half}")
            for c in range(IH // 512):
                for k in range(KT):
                    nc.tensor.matmul(
                        ps[:, 512 * c:512 * (c + 1)],
                        wg[:, k, 128 * s4:128 * (s4 + 1)],
                        xTk[k][:, IH * half + 512 * c:IH * half + 512 * (c + 1)],
                        start=(k == 0), stop=(k == KT - 1))
            nc.vector.tensor_copy(
                out=kTs[s][:, IH * half:IH * (half + 1)], in_=ps)

        def v_tile(pool, tag, wg, vdst, t):
            ps = pool.tile([128, 512], F32, tag=tag, name=f"v_ps{t}")
            for k in range(KT):
                nc.tensor.matmul(
                    ps, xTk[k][:, 128 * t:128 * (t + 1)], wg[:, k, :],
                    start=(k == 0), stop=(k == KT - 1))
            nc.vector.tensor_copy(
                out=vdst[:, t, :, 0:DH],
                in_=ps.rearrange("p (h d) -> p h d", h=8))

        # ---- head-pair emitter with optional per-step filler ----
        def pair(s, ps_av, filler=None):
            av0 = ps_av.tile([DH + 1, IH], F32, tag="av", name=f"av0_{s}")
            av1 = ps_av.tile([DH + 1, IH], F32, tag="av", name=f"av1_{s}")
            avs = [av0, av1]
            step = 0
            for t in range(NT):
                for p in range(2):
                    h = 2 * s + p
                    pb = 64 * p
                    sc = ps_sc.tile([128, IH], F32, tag="sc", name=f"sc{s}_{t}_{p}")
                    for c in range(IH // 512):
                        nc.tensor.matmul(
                            sc[:, 512 * c:512 * (c + 1)],
                            kTs[s][pb:pb + 64, 128 * t:128 * (t + 1)],
                            qTs[s][pb:pb + 64, 512 * c:512 * (c + 1)],
                            start=True, stop=True, tile_position=(pb, 0))
                    at = atp.tile([128, IH], BF16, tag="at", name=f"at{s}_{t}_{p}")
                    nc.scalar.activation(
                        out=at, in_=sc,
                        func=mybir.ActivationFunctionType.Exp, scale=SCALE)
                    vsrc = v_lo if h < 8 else v_hi
                    for c in range(IH // 512):
                        nc.tensor.matmul(
                            avs[p][:, 512 * c:512 * (c + 1)],
                            vsrc[:, t, h % 8, :],
                            at[:, 512 * c:512 * (c + 1)],
                            start=(t == 0), stop=(t == NT - 1))
                    if filler is not None:
                        filler(step)
                    step += 1
            for p in range(2):
                av = avs[p]
                avu = avup.tile([DH + 1, IH], F32, tag="avu", name=f"avu{s}_{p}")
                nc.vector.tensor_copy(out=avu, in_=av)
                rc = rcp.tile([128, IH], BF16, tag="rc", name=f"rc{s}_{p}")
                with nc.allow_low_precision(reason="softmax denom recip in bf16"):
                    nc.vector.reciprocal(
                        out=rc[DH:DH + 1, :], in_=avu[DH:DH + 1, :])
                dr = drbp.tile([IH], BF16, tag="dr", name=f"dr{s}_{p}")
                nc.sync.dma_start(out=dr, in_=rc[DH:DH + 1, :])
                dr_bc = bass.AP(tensor=dr.tensor, offset=dr.offset,
                                ap=[[0, DH]] + [list(dd) for dd in dr.ap])
                bcs = bcsp.tile([DH, IH], BF16, tag="bcs", name=f"bcs{s}_{p}")
                nc.sync.dma_start(out=bcs, in_=dr_bc)
                if p == 0:
                    nc.vector.tensor_mul(
                        out=aoTs[s][0:DH, :], in0=avu[0:DH, :], in1=bcs)
                else:
                    od = oddp.tile([DH, IH], BF16, tag="od", name=f"od{s}")
                    nc.vector.tensor_mul(out=od, in0=avu[0:DH, :], in1=bcs)
                    nc.sync.dma_start(out=aoTs[s][DH:128, :], in_=od)

        # ---------------- phase 1a: v_lo, k0-3, q0-3 ----------------
        p1ctx = ExitStack()
        p1x = p1ctx.enter_context(tc.tile_pool(name="p1_x", bufs=1))
        p1w = p1ctx.enter_context(tc.tile_pool(name="p1_w", bufs=2))
        xTk = [p1x.tile([128, N], BF16, name=f"xTk{k}") for k in range(KT)]
        for k in range(KT):
            nc.sync.dma_start(out=xTk[k], in_=xT_r[:, k, :])

        wgs = {}
        for g in (4, 2, 0, 5, 3, 1):
            wgs[g] = p1w.tile([128, KT, 512], BF16, tag="wg", name=f"wg{g}")
        with tc.tile_pool(name="p1_ps", bufs=2, space="PSUM") as p1ps:
            nc.sync.dma_start(out=wgs[4], in_=w_r[:, :, 2048:2560])
            for t in range(NT):
                v_tile(p1ps, "ps", wgs[4], v_lo, t)
            nc.sync.dma_start(out=wgs[2], in_=w_r[:, :, 1024:1536])
            for s4 in range(4):
                for half in range(2):
                    k_slice(p1ps, "ps", wgs[2], s4, s4, half)
            nc.sync.dma_start(out=wgs[0], in_=w_r[:, :, 0:512])
            for s4 in range(4):
                q_slice(p1ps, "ps", wgs[0], s4, s4)

        # ------------ phase 2 pairs 0-3, with 1b sprinkled ------------
        ps_av = ctx.enter_context(tc.tile_pool(name="ps_av", bufs=2, space="PSUM"))
        if True:
                pair(0, ps_av)

                nc.sync.dma_start(out=wgs[5], in_=w_r[:, :, 2560:3072])

                def fill_v_hi(step):       # 16 v tiles over 32 steps
                    if step % 2 == 0:
                        v_tile(ps_sc, "sc", wgs[5], v_hi, step // 2)
                pair(1, ps_av, fill_v_hi)

                nc.sync.dma_start(out=wgs[3], in_=w_r[:, :, 1536:2048])

                def fill_k_hi(step):       # 8 k slices over 32 steps
                    if step % 4 == 0:
                        i = step // 4
                        k_slice(ps_sc, "sc", wgs[3], i // 2, 4 + i // 2, i % 2)
                pair(2, ps_av, fill_k_hi)

                nc.sync.dma_start(out=wgs[1], in_=w_r[:, :, 512:1024])

                def fill_q_hi(step):       # 4 q slices over 32 steps
                    if step % 8 == 0:
                        i = step // 8
                        q_slice(ps_sc, "sc", wgs[1], i, 4 + i)
                pair(3, ps_av, fill_q_hi)

                p1ctx.close()   # free xT + w staging before pairs 4-7

                for s in range(4, ES):
                    pair(s, ps_av)

                # ---------------- phase 3: output projection ----------------
                with tc.tile_pool(name="p3_st", bufs=2) as p3st, \
                     tc.tile_pool(name="p3_pp", bufs=NPP) as p3pp:
                    pps = []
                    for ns in range(NPP):   # pass 1: heads 0-13 + bias
                        po = ps_av.tile([128, DIM], F32, tag="av", name=f"po{ns}")
                        for c in range(DIM // 512):
                            for hp in range(ES - 1):
                                nc.tensor.matmul(
                                    po[:, 512 * c:512 * (c + 1)],
                                    aoTs[hp][:, 128 * ns:128 * (ns + 1)],
                                    wo_sb[:, hp, 512 * c:512 * (c + 1)],
                                    start=(hp == 0), stop=(hp == ES - 2))
                        pp = p3pp.tile([128, DIM], F32, tag="pp", name=f"pp{ns}")
                        nc.vector.tensor_add(out=pp, in0=po, in1=bias_sb)
                        pps.append(pp)
                    for ns in range(NPP):   # pass 2: join head pair 7
                        po = ps_av.tile([128, DIM], F32, tag="av", name=f"po2_{ns}")
                        for c in range(DIM // 512):
                            nc.tensor.matmul(
                                po[:, 512 * c:512 * (c + 1)],
                                aoTs[ES - 1][:, 128 * ns:128 * (ns + 1)],
                                wo_sb[:, ES - 1, 512 * c:512 * (c + 1)],
                                start=True, stop=True)
                        st = p3st.tile([128, DIM], F32, tag="st", name=f"st{ns}")
                        nc.vector.tensor_add(out=st, in0=po, in1=pps[ns])
                        nc.sync.dma_start(
                            out=out_d.ap()[128 * ns:128 * (ns + 1), :], in_=st)
                    for ns in range(NPP, IH // 128):   # remaining: single pass
                        po = ps_av.tile([128, DIM], F32, tag="av", name=f"po1_{ns}")
                        for c in range(DIM // 512):
                            for hp in range(ES):
                                nc.tensor.matmul(
                                    po[:, 512 * c:512 * (c + 1)],
                                    aoTs[hp][:, 128 * ns:128 * (ns + 1)],
                                    wo_sb[:, hp, 512 * c:512 * (c + 1)],
                                    start=(hp == 0), stop=(hp == ES - 1))
                        st = p3st.tile([128, DIM], F32, tag="st", name=f"st{ns}")
                        nc.vector.tensor_add(out=st, in0=po, in1=bias_sb)
                        nc.sync.dma_start(
                            out=out_d.ap()[128 * ns:128 * (ns + 1), :], in_=st)


def get_program():
    if "nc" not in _CACHE:
        _CACHE["nc"] = _build_program()
    return _CACHE["nc"]


def make_in_maps(x, w_qkv, w_out, b_out):
    bf = ml_dtypes.bfloat16
    w_qkv_b = np.ascontiguousarray(w_qkv, np.float32).astype(bf)
    w_out_b = np.ascontiguousarray(w_out, np.float32).astype(bf)
    b_out_f = np.ascontiguousarray(b_out, np.float32)
    in_maps = []
    for core in range(NCORES):
        b, half = core // 2, core % 2
        xT = np.ascontiguousarray(np.asarray(x[b], np.float32).T).astype(bf)
        if half == 1:   # rotate this core's query half to the front
            xT = np.concatenate([xT[:, IH:], xT[:, :IH]], axis=1)
        in_maps.append({
            "xT": np.ascontiguousarray(xT),
            "w_qkv": w_qkv_b,
            "w_out": w_out_b,
            "b_out": b_out_f,
        })
    return in_maps


def kernel(x, w_qkv, w_out, b_out):
    nc = get_program()
    in_maps = make_in_maps(x, w_qkv, w_out, b_out)
    res = bass_utils.run_bass_kernel_spmd(nc, in_maps, core_ids=list(range(NCORES)))
    out = np.empty((B, N, DIM), np.float32)
    for core in range(NCORES):
        b, half = core // 2, core % 2
        out[b, IH * half:IH * (half + 1), :] = res.results[core]["out"]
    return out


# revision 23
# speedup vs baseline: 1.2686x; 1.2686x over previous
"""Trainium2 Bass kernel for multi-head attention (B=4, N=2048, DIM=1024, H=16, DH=64).

Sharding: 8 cores = 4 batches x 2 query-halves. Each core receives x[b]^T with
its query-half columns rotated to the front (attention is invariant to a
consistent permutation of the key/value axis), computes q for columns 0:1024,
k/v for all 2048, runs scores^T = k_h^T @ q_h per head (row-tiled pairs),
softmax via exp + ones-column denominator folded into the AV matmul, and the
full output projection for its rows. Outputs are disjoint across cores.

Emission interleaves the second half of the projections into the first head
pairs (PE executes in program order, so overlap must be authored), and the
output projection runs two-pass (heads 0-6 early, head pair 7 joined late) to
hide the final softmax-normalize latency.
"""

import os

import numpy as np
import ml_dtypes

import concourse.bass as bass
import concourse.tile as tile
from concourse import bacc, mybir
from concourse import bass_utils

B, N, DIM = 4, 2048, 1024
HEADS, DH = 16, 64
INNER = HEADS * DH
SCALE = DH ** -0.5
NCORES = 8
IH = N // 2          # query rows per core (i-half)
BF16 = mybir.dt.bfloat16
F32 = mybir.dt.float32

KT = DIM // 128          # 8 contraction tiles for projections
NT = N // 128            # 16 j tiles
ES = INNER // 128        # 8 e-slices for q or k
NPP = 8                  # ns groups computed two-pass in phase 3 (all)

_CACHE = {}


def _build_program():
    nc = bacc.Bacc("TRN2", target_bir_lowering=False, debug=False)

    xT_d = nc.dram_tensor("xT", [DIM, N], BF16, kind="ExternalInput")
    wqkv_d = nc.dram_tensor("w_qkv", [DIM, 3 * INNER], BF16, kind="ExternalInput")
    wout_d = nc.dram_tensor("w_out", [INNER, DIM], BF16, kind="ExternalInput")
    bout_d = nc.dram_tensor("b_out", [DIM], F32, kind="ExternalInput")
    out_d = nc.dram_tensor("out", [IH, DIM], F32, kind="ExternalOutput")

    with tile.TileContext(nc) as tc:
        _emit(tc, nc, xT_d, wqkv_d, wout_d, bout_d, out_d)
    nc.compile()
    return nc


def _emit(tc, nc, xT_d, wqkv_d, wout_d, bout_d, out_d):
    from contextlib import ExitStack

    xT_r = xT_d.ap().rearrange("(t p) n -> p t n", p=128)       # [128, 8, 2048]
    w_r = wqkv_d.ap().rearrange("(t p) e -> p t e", p=128)      # [128, 8, 3072]
    wo_r = wout_d.ap().rearrange("(t p) d -> p t d", p=128)     # [128, 8, 1024]

    bap = bout_d.ap()
    bias_bcast = bass.AP(tensor=bap.tensor, offset=bap.offset,
                         ap=[[0, 128]] + [list(d) for d in bap.ap])

    with ExitStack() as ctx:
        consts = ctx.enter_context(tc.tile_pool(name="consts", bufs=1))
        qkv_out = ctx.enter_context(tc.tile_pool(name="qkv_out", bufs=1))
        attn_out = ctx.enter_context(tc.tile_pool(name="attn_out", bufs=1))
        atp = ctx.enter_context(tc.tile_pool(name="attnT", bufs=4))
        rcp = ctx.enter_context(tc.tile_pool(name="rcp", bufs=1))
        bcsp = ctx.enter_context(tc.tile_pool(name="bcs", bufs=2))
        avup = ctx.enter_context(tc.tile_pool(name="avu", bufs=2))
        oddp = ctx.enter_context(tc.tile_pool(name="odd", bufs=1))
        drbp = ctx.enter_context(tc.tile_pool(name="drb", bufs=2, space="DRAM"))
        ps_sc = ctx.enter_context(tc.tile_pool(name="ps_sc", bufs=2, space="PSUM"))

        bias_sb = consts.tile([128, DIM], F32)
        nc.sync.dma_start(out=bias_sb, in_=bias_bcast)
        wo_sb = consts.tile([128, ES, DIM], BF16)       # head pair hp at [:, hp, :]
        nc.sync.dma_start(out=wo_sb, in_=wo_r)

        qTs = [qkv_out.tile([128, IH], BF16, name=f"qT{s}") for s in range(ES)]
        kTs = [qkv_out.tile([128, N], BF16, name=f"kT{s}") for s in range(ES)]
        v_lo = qkv_out.tile([128, NT, 8, DH + 1], BF16)  # heads 0-7 (+ones col)
        v_hi = qkv_out.tile([128, NT, 8, DH + 1], BF16)  # heads 8-15
        nc.vector.memset(v_lo[:, :, :, DH], 1.0)
        nc.vector.memset(v_hi[:, :, :, DH], 1.0)
        aoTs = [attn_out.tile([128, IH], BF16, name=f"aoT{s}") for s in range(ES)]

        # ---- projection group emitters (psum from a given pool/tag) ----
        def q_slice(pool, tag, wg, s4, s):
            ps = pool.tile([128, IH], F32, tag=tag, name=f"q_ps{s}")
            for c in range(IH // 512):
                for k in range(KT):
                    nc.tensor.matmul(
                        ps[:, 512 * c:512 * (c + 1)],
                        wg[:, k, 128 * s4:128 * (s4 + 1)],
                        xTk[k][:, 512 * c:512 * (c + 1)],
                        start=(k == 0), stop=(k == KT - 1))
            nc.vector.tensor_copy(out=qTs[s], in_=ps)

        def k_slice(pool, tag, wg, s4, s, half):
            ps = pool.tile([128, IH], F32, tag=tag, name=f"k_ps{s}_{half}")
            for c in range(IH // 512):
                for k in range(KT):
                    nc.tensor.matmul(
                        ps[:, 512 * c:512 * (c + 1)],
                        wg[:, k, 128 * s4:128 * (s4 + 1)],
                        xTk[k][:, IH * half + 512 * c:IH * half + 512 * (c + 1)],
                        start=(k == 0), stop=(k == KT - 1))
            nc.vector.tensor_copy(
                out=kTs[s][:, IH * half:IH * (half + 1)], in_=ps)

        def v_tile(pool, tag, wg, vdst, t):
            ps = pool.tile([128, 512], F32, tag=tag, name=f"v_ps{t}")
            for k in range(KT):
                nc.tensor.matmul(
                    ps, xTk[k][:, 128 * t:128 * (t + 1)], wg[:, k, :],
                    start=(k == 0), stop=(k == KT - 1))
            nc.vector.tensor_copy(
                out=vdst[:, t, :, 0:DH],
                in_=ps.rearrange("p (h d) -> p h d", h=8))

        # ---- head-pair emitter with optional per-step filler ----
        def pair(s, ps_av, filler=None):
            av0 = ps_av.tile([DH + 1, IH], F32, tag="av", name=f"av0_{s}")
            av1 = ps_av.tile([DH + 1, IH], F32, tag="av", name=f"av1_{s}")
            avs = [av0, av1]
            step = 0
            for t in range(NT):
                for p in range(2):
                    h = 2 * s + p
                    pb = 64 * p
                    sc = ps_sc.tile([128, IH], F32, tag="sc", name=f"sc{s}_{t}_{p}")
                    for c in range(IH // 512):
                        nc.tensor.matmul(
                            sc[:, 512 * c:512 * (c + 1)],
                            kTs[s][pb:pb + 64, 128 * t:128 * (t + 1)],
                            qTs[s][pb:pb + 64, 512 * c:512 * (c + 1)],
                            start=True, stop=True, tile_position=(pb, 0))
                    at = atp.tile([128, IH], BF16, tag="at", name=f"at{s}_{t}_{p}")
                    nc.scalar.activation(
                        out=at, in_=sc,
                        func=mybir.ActivationFunctionType.Exp, scale=SCALE)
                    vsrc = v_lo if h < 8 else v_hi
                    for c in range(IH // 512):
                        nc.tensor.matmul(
                            avs[p][:, 512 * c:512 * (c + 1)],
                            vsrc[:, t, h % 8, :],
                            at[:, 512 * c:512 * (c + 1)],
                            start=(t == 0), stop=(t == NT - 1))
                    if filler is not None:
                        filler(step)
                    step += 1
            for p in range(2):
                av = avs[p]
                avu = avup.tile([DH + 1, IH], F32, tag="avu", name=f"avu{s}_{p}")
                nc.vector.tensor_copy(out=avu, in_=av)
                rc = rcp.tile([128, IH], BF16, tag="rc", name=f"rc{s}_{p}")
                with nc.allow_low_precision(reason="softmax denom recip in bf16"):
                    nc.vector.reciprocal(
                        out=rc[DH:DH + 1, :], in_=avu[DH:DH + 1, :])
                dr = drbp.tile([IH], BF16, tag="dr", name=f"dr{s}_{p}")
                nc.sync.dma_start(out=dr, in_=rc[DH:DH + 1, :])
                dr_bc = bass.AP(tensor=dr.tensor, offset=dr.offset,
                                ap=[[0, DH]] + [list(dd) for dd in dr.ap])
                bcs = bcsp.tile([DH, IH], BF16, tag="bcs", name=f"bcs{s}_{p}")
                nc.sync.dma_start(out=bcs, in_=dr_bc)
                if p == 0:
                    nc.vector.tensor_mul(
                        out=aoTs[s][0:DH, :], in0=avu[0:DH, :], in1=bcs)
                else:
                    od = oddp.tile([DH, IH], BF16, tag="od", name=f"od{s}")
                    nc.vector.tensor_mul(out=od, in0=avu[0:DH, :], in1=bcs)
                    nc.sync.dma_start(out=aoTs[s][DH:128, :], in_=od)

        # ---------------- phase 1a: v_lo, k0-3, q0-3 ----------------
        p1ctx = ExitStack()
        p1x = p1ctx.enter_context(tc.tile_pool(name="p1_x", bufs=1))
        p1w = p1ctx.enter_context(tc.tile_pool(name="p1_w", bufs=2))
        xTk = [p1x.tile([128, N], BF16, name=f"xTk{k}") for k in range(KT)]
        for k in range(KT):
            eng = nc.sync if k % 2 == 0 else nc.gpsimd
            eng.dma_start(out=xTk[k], in_=xT_r[:, k, :])

        wgs = {}
        for g in (4, 2, 0, 5, 3, 1):
            wgs[g] = p1w.tile([128, KT, 512], BF16, tag="wg", name=f"wg{g}")
        with tc.tile_pool(name="p1_ps", bufs=2, space="PSUM") as p1ps:
            nc.sync.dma_start(out=wgs[4], in_=w_r[:, :, 2048:2560])
            for t in range(NT):
                v_tile(p1ps, "ps", wgs[4], v_lo, t)
            nc.sync.dma_start(out=wgs[2], in_=w_r[:, :, 1024:1536])
            for s4 in range(4):
                for half in range(2):
                    k_slice(p1ps, "ps", wgs[2], s4, s4, half)
            nc.sync.dma_start(out=wgs[0], in_=w_r[:, :, 0:512])
            for s4 in range(4):
                q_slice(p1ps, "ps", wgs[0], s4, s4)

        # ------------ phase 2 pairs 0-3, with 1b sprinkled ------------
        ps_av = ctx.enter_context(tc.tile_pool(name="ps_av", bufs=2, space="PSUM"))
        if True:
                pair(0, ps_av)

                pair(1, ps_av)

                # second half of projections between pairs 1 and 2
                # (borrows the idle scores-psum slots; no QK runs during this block)
                nc.sync.dma_start(out=wgs[5], in_=w_r[:, :, 2560:3072])
                for t in range(NT):
                    v_tile(ps_sc, "sc", wgs[5], v_hi, t)
                nc.sync.dma_start(out=wgs[3], in_=w_r[:, :, 1536:2048])
                for s4 in range(4):
                    for half in range(2):
                        k_slice(ps_sc, "sc", wgs[3], s4, 4 + s4, half)
                nc.sync.dma_start(out=wgs[1], in_=w_r[:, :, 512:1024])
                for s4 in range(4):
                    q_slice(ps_sc, "sc", wgs[1], s4, 4 + s4)

                for s in range(2, 4):
                    pair(s, ps_av)

                p1ctx.close()   # free xT + w staging before pairs 4-7

                for s in range(4, ES):
                    pair(s, ps_av)

                # ---------------- phase 3: output projection ----------------
                with tc.tile_pool(name="p3_st", bufs=2) as p3st, \
                     tc.tile_pool(name="p3_pp", bufs=NPP) as p3pp:
                    pps = []
                    for ns in range(NPP):   # pass 1: heads 0-13 + bias
                        po = ps_av.tile([128, DIM], F32, tag="av", name=f"po{ns}")
                        for c in range(DIM // 512):
                            for hp in range(ES - 1):
                                nc.tensor.matmul(
                                    po[:, 512 * c:512 * (c + 1)],
                                    aoTs[hp][:, 128 * ns:128 * (ns + 1)],
                                    wo_sb[:, hp, 512 * c:512 * (c + 1)],
                                    start=(hp == 0), stop=(hp == ES - 2))
                        pp = p3pp.tile([128, DIM], F32, tag="pp", name=f"pp{ns}")
                        nc.vector.tensor_add(out=pp, in0=po, in1=bias_sb)
                        pps.append(pp)
                    for ns in range(NPP):   # pass 2: join head pair 7
                        po = ps_av.tile([128, DIM], F32, tag="av", name=f"po2_{ns}")
                        for c in range(DIM // 512):
                            nc.tensor.matmul(
                                po[:, 512 * c:512 * (c + 1)],
                                aoTs[ES - 1][:, 128 * ns:128 * (ns + 1)],
                                wo_sb[:, ES - 1, 512 * c:512 * (c + 1)],
                                start=True, stop=True)
                        st = p3st.tile([128, DIM], F32, tag="st", name=f"st{ns}")
                        nc.vector.tensor_add(out=st, in0=po, in1=pps[ns])
                        nc.sync.dma_start(
                            out=out_d.ap()[128 * ns:128 * (ns + 1), :], in_=st)
                    for ns in range(NPP, IH // 128):   # remaining: single pass
                        po = ps_av.tile([128, DIM], F32, tag="av", name=f"po1_{ns}")
                        for c in range(DIM // 512):
                            for hp in range(ES):
                                nc.tensor.matmul(
                                    po[:, 512 * c:512 * (c + 1)],
                                    aoTs[hp][:, 128 * ns:128 * (ns + 1)],
                                    wo_sb[:, hp, 512 * c:512 * (c + 1)],
                                    start=(hp == 0), stop=(hp == ES - 1))
                        st = p3st.tile([128, DIM], F32, tag="st", name=f"st{ns}")
                        nc.vector.tensor_add(out=st, in0=po, in1=bias_sb)
                        nc.sync.dma_start(
                            out=out_d.ap()[128 * ns:128 * (ns + 1), :], in_=st)


def get_program():
    if "nc" not in _CACHE:
        _CACHE["nc"] = _build_program()
    return _CACHE["nc"]


def make_in_maps(x, w_qkv, w_out, b_out):
    bf = ml_dtypes.bfloat16
    w_qkv_b = np.ascontiguousarray(w_qkv, np.float32).astype(bf)
    w_out_b = np.ascontiguousarray(w_out, np.float32).astype(bf)
    b_out_f = np.ascontiguousarray(b_out, np.float32)
    in_maps = []
    for core in range(NCORES):
        b, half = core // 2, core % 2
        xT = np.ascontiguousarray(np.asarray(x[b], np.float32).T).astype(bf)
        if half == 1:   # rotate this core's query half to the front
            xT = np.concatenate([xT[:, IH:], xT[:, :IH]], axis=1)
        in_maps.append({
            "xT": np.ascontiguousarray(xT),
            "w_qkv": w_qkv_b,
            "w_out": w_out_b,
            "b_out": b_out_f,
        })
    return in_maps


def kernel(x, w_qkv, w_out, b_out):
    nc = get_program()
    in_maps = make_in_maps(x, w_qkv, w_out, b_out)
    res = bass_utils.run_bass_kernel_spmd(nc, in_maps, core_ids=list(range(NCORES)))
    out = np.empty((B, N, DIM), np.float32)
    for core in range(NCORES):
        b, half = core // 2, core % 2
        out[b, IH * half:IH * (half + 1), :] = res.results[core]["out"]
    return out
